# revision 9
# baseline (speedup 1.0000x reference)
"""CenterLoss kernel for Trainium2 (8 NeuronCores, sharded by class range).

Algorithm
---------
reference computes:
    counts[c] = #{i: y_i = c};  sums[c] = sum_{i: y_i = c} f_i
    means = sums / max(counts, 1);  present = counts > 0
    n_c = present ? 0.5*centers_c + 0.5*means_c : centers_c
    loss = 0.5 * mean_i ||f_i - n_{y_i}||^2

Expanding the loss (every class that appears in the batch is present):
    B * 2 * loss = S1 - 0.5*A - 0.75*X + 0.25*W
where
    S1 = sum_i ||f_i||^2
    A  = sum_c sums_c . centers_c
    X  = sum_{c present} ||sums_c||^2 / counts_c
    W  = sum_c counts_c * ||centers_c||^2

Distribution: segment_reduce is sharded BY SEGMENT ID. The host splits the
1000 classes into 8 contiguous ranges with ~equal row counts (quantiles of
the label histogram) and routes each row to the core owning its class.  Each
core then reduces over <=127 local classes, so its one-hot is only 128 wide
(vs 1024 for the data-parallel split) -- 8x less one-hot and matmul work.

Per core the device computes, over its <=17408 routed rows (fp8 feats):
  - seg sums+counts: one-hot [128rows, 2, 128cls] fp8 built on DVE/GPSIMD,
    contracted with fp8 feats pairs via DoubleRow fp8 matmuls (PE, 2 row
    tiles per instruction) into a PSUM bank [128, 257]
  - S1 partials: two fp8 DoubleRow Gram matmuls per pair (f8.T @ f8 for the
    two 128-col halves); the host reads the diagonals
The host sums per-core partials and evaluates the tiny [C,D] closed form
above (the gather/unshard step).  Inputs whose label distribution is too
skewed for the class split (a range needing >127 classes or >17408 rows)
fall back to the data-parallel fp16 one-hot kernel below.
"""

import sys

sys.path.insert(0, "/opt/trn_rl_repo")

import numpy as np

# problem shape (hardcoded per the harness contract)
B, D, C = 131072, 256, 1000
N_CORES = 8
P = 128

# ---- fast path (class-range sharding, fp8 DoubleRow) ----
TILES_F = 128          # row tiles of 128 per core (exactly B/8 rows, no pads)
PAIRS = TILES_F // 2   # DoubleRow processes two row tiles per matmul
ROWS_F = TILES_F * P   # 16384 = B / N_CORES exactly
CLS_CAP = 128          # max classes per core window (else fallback)
NFREE = D + 1          # 256 feat cols + 1 counts col in the output
FROW = 512             # fp8 bytes per (pair, partition): [half, plane, 128]
NOUT = NFREE + 2 * P   # seg [128,257] + gram0 [128,128] + gram1 [128,128]

_CACHE: dict = {}


def _build_fast():
    import concourse.bacc as bacc
    import concourse.bass as bass
    from concourse import mybir
    from concourse.tile import TileContext

    nc = bacc.Bacc("TRN2", target_bir_lowering=False)

    f8 = mybir.dt.float8e4
    # per (pair, partition): 512B = [half, plane, 128 cols] so DoubleRow
    # weight reads (Gram matmuls) see contiguous [K, 2, 128] blocks
    feats_in = nc.dram_tensor(
        "feats8", [PAIRS * P * FROW], f8, kind="ExternalInput"
    )
    labels_in = nc.dram_tensor(
        "labels", [P, TILES_F], mybir.dt.float32, kind="ExternalInput"
    )
    out_t = nc.dram_tensor("out", [P, NOUT], mybir.dt.float32, kind="ExternalOutput")

    # feats load batching: small first loads so the pipeline starts early,
    # small last loads so the final DMA-completion sem (+900ns) only gates a
    # couple of pairs of matmuls
    load_sizes = [2, 6, 8, 8, 8, 8, 8, 8, 5, 2, 1]
    assert sum(load_sizes) == PAIRS

    with TileContext(nc) as tc:
        with (
            tc.tile_pool(name="const", bufs=1) as const,
            tc.tile_pool(name="fin", bufs=6) as fin,
            tc.tile_pool(name="ohp", bufs=24) as ohp,
            tc.tile_pool(name="evp", bufs=1) as evp,
            tc.tile_pool(name="psp", bufs=1, space="PSUM") as psp,
        ):
            # labels arrive as fp32 [P, TILES] (direct is_equal scalars);
            # iota 0..127 built on GPSIMD directly in fp16 (exact <= 2048)
            labels_t = const.tile([P, TILES_F], mybir.dt.float32, tag="labels_t")
            nc.sync.dma_start(out=labels_t[:], in_=labels_in[:])
            iota_i = const.tile([P, P], mybir.dt.int32, tag="iota_i")
            nc.gpsimd.iota(iota_i[:], pattern=[[1, P]], channel_multiplier=0)
            iota_t = const.tile([P, P], mybir.dt.float16, tag="iota_t")
            nc.vector.tensor_copy(out=iota_t[:], in_=iota_i[:])
            ones_c = const.tile([P, 2, 1], f8, tag="ones_c")
            nc.vector.memset(ones_c[:], 1.0)

            ps_lo = psp.tile([P, P], mybir.dt.float32, tag="ps_lo")
            ps_hi = psp.tile([P, P], mybir.dt.float32, tag="ps_hi")
            ps_cnt = psp.tile([P, 1], mybir.dt.float32, tag="ps_cnt")
            ps_g0 = psp.tile([P, P], mybir.dt.float32, tag="ps_g0")
            ps_g1 = psp.tile([P, P], mybir.dt.float32, tag="ps_g1")

            # HAM warm-up: keep the PE continuously busy from ~0.4us so the
            # 3us p-state ramp completes before the real stream begins and the
            # first real matmuls run at full clock. Results are discarded by
            # the start=True PSUM reset of the real pair-0 matmuls.
            warm_l = const.tile([P, 2, P], f8, tag="warm_l")
            nc.vector.memset(warm_l[:1, :, :1], 0.0)
            for _ in range(42):
                nc.tensor.matmul(
                    out=ps_lo[:],
                    lhsT=warm_l[:],
                    rhs=warm_l[:],
                    start=True,
                    stop=True,
                    perf_mode=mybir.MatmulPerfMode.DoubleRow,
                )

            DR = mybir.MatmulPerfMode.DoubleRow
            j = 0  # global pair index
            for li, g in enumerate(load_sizes):
                fg = fin.tile([P, g, 2, 2, P], f8, tag="fg", name="fg")
                # alternate HWDGE rings (SP / ACT) so descriptor generation
                # of load k+1 overlaps load k's transfer
                dma_eng = nc.sync if li % 2 == 0 else nc.scalar
                dma_eng.dma_start(
                    out=fg[:],
                    in_=bass.AP(
                        tensor=feats_in[:].tensor,
                        offset=j * P * FROW,
                        ap=[[FROW, P], [P * FROW, g], [1, FROW]],
                    ),
                )
                for jj in range(g):
                    # one-hot [128 rows, 2 planes, 128 classes] in fp8;
                    # a slice of pairs on GPSIMD to offload the DVE
                    oh = ohp.tile([P, 2, P], f8, tag="oh")
                    eng = nc.gpsimd if j % 16 < 5 else nc.vector
                    for i in range(2):
                        t = 2 * j + i
                        eng.tensor_scalar(
                            oh[:, i, :],
                            iota_t[:],
                            labels_t[:, t : t + 1],
                            None,
                            mybir.AluOpType.is_equal,
                        )
                    start = j == 0
                    stop = j == PAIRS - 1
                    f_lo = fg[:, jj, 0, :, :]
                    f_hi = fg[:, jj, 1, :, :]
                    nc.tensor.matmul(out=ps_lo[:], lhsT=oh[:], rhs=f_lo,
                                     start=start, stop=stop, perf_mode=DR)
                    nc.tensor.matmul(out=ps_hi[:], lhsT=oh[:], rhs=f_hi,
                                     start=start, stop=stop, perf_mode=DR)
                    nc.tensor.matmul(out=ps_cnt[:], lhsT=oh[:], rhs=ones_c[:],
                                     start=start, stop=stop, perf_mode=DR)
                    # S1 partials: Gram of the two 128-col halves; the diag
                    # is extracted below; feats-only, no one-hot dep
                    nc.tensor.matmul(out=ps_g0[:], lhsT=f_lo, rhs=f_lo,
                                     start=start, stop=stop, perf_mode=DR)
                    nc.tensor.matmul(out=ps_g1[:], lhsT=f_hi, rhs=f_hi,
                                     start=start, stop=stop, perf_mode=DR)
                    j += 1

            # evacuate PSUM -> SBUF split across the idle ACT and DVE
            # engines; ship the full Gram banks (host reads the diagonals).
            # Two stores on separate HWDGE rings so the seg store's issue
            # chain overlaps the gram evacuation.
            ev = evp.tile([P, NOUT], mybir.dt.float32, tag="ev")
            nc.scalar.copy(out=ev[:, 0:P], in_=ps_lo[:])
            nc.vector.tensor_copy(out=ev[:, P : 2 * P], in_=ps_hi[:])
            nc.scalar.copy(out=ev[:, 2 * P : NFREE], in_=ps_cnt[:])
            nc.sync.dma_start(out=out_t[:, 0:NFREE], in_=ev[:, 0:NFREE])
            nc.vector.tensor_copy(out=ev[:, NFREE : NFREE + P], in_=ps_g1[:])
            nc.scalar.copy(out=ev[:, NFREE + P : NOUT], in_=ps_g0[:])
            nc.scalar.dma_start(out=out_t[:, NFREE:NOUT], in_=ev[:, NFREE:NOUT])

    nc.compile()
    return nc


# ---- fallback path (data-parallel over B, fp16 one-hot matmuls) ----
BS = B // N_CORES  # 16384 rows per core
G = 4  # row-tiles per DMA group
TILES = BS // P  # 128
GROUPS = TILES // G  # 32
CPAD = 1024  # padded class count
CCHUNKS = CPAD // P  # 8
FB_FSTRIDE = 264  # fp16 sub-tile stride (4B aligned, 16B padded)
TAILG = 4  # trailing groups processed chunk-outer (store/compute overlap)


def _build_fallback():
    import concourse.bacc as bacc
    import concourse.bass as bass
    from concourse import mybir
    from concourse.tile import TileContext

    nc = bacc.Bacc("TRN2", target_bir_lowering=False)

    feats = nc.dram_tensor("feats", [BS, D], mybir.dt.float32, kind="ExternalInput")
    labels_in = nc.dram_tensor(
        "labels", [P, TILES], mybir.dt.float16, kind="ExternalInput"
    )
    out_sums = nc.dram_tensor(
        "out_sums", [P, CCHUNKS * NFREE + 1], mybir.dt.float32, kind="ExternalOutput"
    )

    feats_ap = feats[:]

    with TileContext(nc) as tc:
        with (
            tc.tile_pool(name="const", bufs=1) as const,
            tc.tile_pool(name="fin", bufs=4) as fin,
            tc.tile_pool(name="sq", bufs=2) as sqp,
            tc.tile_pool(name="f16p", bufs=TAILG + 2) as f16p,
            tc.tile_pool(name="ohp", bufs=4 * TAILG + 6) as ohp,
            tc.tile_pool(name="accp", bufs=1) as accp,
            tc.tile_pool(name="psp", bufs=1, space="PSUM") as psp,
        ):
            labels16_t = const.tile([P, TILES], mybir.dt.float16, tag="labels16_t")
            nc.sync.dma_start(out=labels16_t[:], in_=labels_in[:])
            labels_t = const.tile([P, TILES], mybir.dt.float32, tag="labels_t")
            nc.vector.tensor_copy(out=labels_t[:], in_=labels16_t[:])
            iota_i = const.tile([P, CPAD], mybir.dt.int32, tag="iota_i")
            nc.gpsimd.iota(iota_i[:], pattern=[[1, CPAD]], channel_multiplier=0)
            iota_f = const.tile([P, CPAD], mybir.dt.float16, tag="iota_f")
            nc.vector.tensor_copy(out=iota_f[:], in_=iota_i[:])
            iota_t = iota_f[:]

            tail_ohs, tail_f16gs = [], []
            s1cols = accp.tile([P, GROUPS + 3], mybir.dt.float32, tag="s1cols")
            s1_extra_col = [GROUPS]
            psums = [
                psp.tile(
                    [P, NFREE], mybir.dt.float32, tag=f"psum{k}", name=f"psum{k}"
                )
                for k in range(CCHUNKS)
            ]
            warm = const.tile([P, NFREE], mybir.dt.float16, tag="warm")
            nc.vector.memset(warm[:1, :1], 0.0)
            for w in range(12):
                nc.tensor.matmul(
                    out=psums[0][:],
                    lhsT=warm[:, 0:P],
                    rhs=warm[:],
                    start=True,
                    stop=True,
                )

            for t in range(GROUPS):
                f16g = f16p.tile([P, G, FB_FSTRIDE], mybir.dt.float16, tag="f16g")
                if t == 0:
                    halves = ((0, 1), (1, 1), (2, 2))
                elif t == 1:
                    halves = ((0, 2), (2, 2))
                else:
                    halves = ((0, G),)
                for h, (off, gh) in enumerate(halves):
                    fg = fin.tile([P, gh, D], mybir.dt.float32, tag="fg", name="fg")
                    dma_eng = nc.scalar if t == 0 else nc.sync
                    dma_eng.dma_start(
                        out=fg[:],
                        in_=bass.AP(
                            tensor=feats_ap.tensor,
                            offset=(t * G + off) * P * D,
                            ap=[[D, P], [P * D, gh], [1, D]],
                        ),
                    )
                    nc.scalar.copy(out=f16g[:, off : off + gh, 0:D], in_=fg[:])
                    sqt = sqp.tile([P, gh, D], mybir.dt.float32, tag="sqt", name="sqt")
                    if h == 0:
                        col = t
                    else:
                        col = s1_extra_col[0]
                        s1_extra_col[0] += 1
                    nc.scalar.activation(
                        out=sqt[:],
                        in_=fg[:],
                        func=mybir.ActivationFunctionType.Square,
                        accum_out=s1cols[:, col : col + 1],
                    )
                nc.vector.memset(f16g[:, :, D : D + 1], 1.0)

                ohs = []
                for s in range(G):
                    jj = t * G + s
                    oh = ohp.tile([P, CPAD], mybir.dt.float16, tag="oh")
                    nc.vector.tensor_scalar(
                        oh[:],
                        iota_t,
                        labels_t[:, jj : jj + 1],
                        None,
                        mybir.AluOpType.is_equal,
                    )
                    ohs.append(oh)
                if t < GROUPS - TAILG:
                    for s in range(G):
                        rhs = f16g[:, s, 0:NFREE]
                        for k in range(CCHUNKS):
                            nc.tensor.matmul(
                                out=psums[k][:],
                                lhsT=ohs[s][:, k * P : (k + 1) * P],
                                rhs=rhs,
                                start=(t == 0 and s == 0),
                                stop=False,
                            )
                else:
                    tail_ohs.append(ohs)
                    tail_f16gs.append(f16g)
            for k in range(CCHUNKS):
                for gg, (ohs_g, f16g_g) in enumerate(zip(tail_ohs, tail_f16gs)):
                    for s in range(G):
                        nc.tensor.matmul(
                            out=psums[k][:],
                            lhsT=ohs_g[s][:, k * P : (k + 1) * P],
                            rhs=f16g_g[:, s, 0:NFREE],
                            start=False,
                            stop=(gg == TAILG - 1 and s == G - 1),
                        )

            ev = accp.tile([P, CCHUNKS * NFREE + 1], mybir.dt.float32, tag="ev")
            nc.vector.tensor_reduce(
                out=ev[:, CCHUNKS * NFREE : CCHUNKS * NFREE + 1],
                in_=s1cols[:],
                axis=mybir.AxisListType.X,
                op=mybir.AluOpType.add,
            )
            for k in range(CCHUNKS):
                dst = ev[:, k * NFREE : (k + 1) * NFREE]
                if k % 2 == 0:
                    nc.vector.tensor_copy(out=dst, in_=psums[k][:])
                else:
                    nc.scalar.copy(out=dst, in_=psums[k][:])
            for k in range(CCHUNKS):
                lo = k * NFREE
                hi = (k + 1) * NFREE + (1 if k == CCHUNKS - 1 else 0)
                nc.sync.dma_start(out=out_sums[:, lo:hi], in_=ev[:, lo:hi])

    nc.compile()
    return nc


def _get_program(which="fast"):
    key = f"nc_{which}"
    if key not in _CACHE:
        _CACHE[key] = _build_fast() if which == "fast" else _build_fallback()
    return _CACHE[key]


def _run_spmd(nc, in_maps, trace=False):
    from concourse.bass_utils import run_bass_kernel_spmd

    kw = {"trace": True} if trace else {}
    try:
        return run_bass_kernel_spmd(nc, in_maps, core_ids=list(range(N_CORES)), **kw)
    except Exception:
        import time

        time.sleep(2.0)
        return run_bass_kernel_spmd(nc, in_maps, core_ids=list(range(N_CORES)), **kw)


def _class_split(labels_i):
    """Sort rows by class and cut into 8 exact ROWS_F-row shards.

    A class may straddle two shards (both report partial sums; the host
    adds).  Returns (order, windows) where windows[k] = (w0, ncls) is the
    local class window of shard k, or None if any window exceeds CLS_CAP
    (pathologically spread labels).
    """
    order = np.argsort(labels_i, kind="stable")
    sl = labels_i[order]
    windows = []
    for k in range(N_CORES):
        w0 = int(sl[k * ROWS_F])
        w1 = int(sl[(k + 1) * ROWS_F - 1])
        if w1 - w0 + 1 > CLS_CAP:
            return None
        windows.append((w0, w1 - w0 + 1))
    return order, windows


def _kernel_fast(feats, centers, labels_i, order, windows, trace=False):
    import ml_dtypes

    f8dt = ml_dtypes.float8_e4m3
    feats8 = feats.astype(f8dt)

    in_maps = []
    for k in range(N_CORES):
        idx = order[k * ROWS_F : (k + 1) * ROWS_F]
        w0, _ = windows[k]
        arr = feats8[idx]
        # [row, col] -> [pair, partition, half, plane, 128] (512B contiguous
        # per (pair, partition) so DoubleRow weight reads are contiguous)
        buf = np.ascontiguousarray(
            arr.reshape(PAIRS, 2, P, 2, P).transpose(0, 2, 3, 1, 4)
        )
        lab = (labels_i[idx] - w0).astype(np.float32)
        ltile = np.ascontiguousarray(lab.reshape(TILES_F, P).T)
        in_maps.append({"feats8": buf.reshape(-1), "labels": ltile})

    nc = _get_program("fast")
    _CACHE["nc"] = nc  # last-used program, for test harness introspection
    res = _run_spmd(nc, in_maps, trace=trace)

    sums = np.zeros((C, D), dtype=np.float64)
    counts = np.zeros(C, dtype=np.float64)
    S1 = 0.0
    for k, (w0, ncls) in enumerate(windows):
        raw = res.results[k]["out"]
        # out cols: [0:128]=sums lo-half, [128:256]=sums hi-half, [256]=counts,
        # [257:385]=gram1, [385:513]=gram0 (S1 = sum of their diagonals)
        sums[w0 : w0 + ncls] += raw[:ncls, :D].astype(np.float64)
        counts[w0 : w0 + ncls] += raw[:ncls, D].astype(np.float64)
        S1 += float(np.trace(raw[:, NFREE : NFREE + P]))
        S1 += float(np.trace(raw[:, NFREE + P : NOUT]))

    c64 = centers.astype(np.float64)
    A = float((sums * c64).sum())
    present = counts > 0
    X = float((np.square(sums).sum(axis=1)[present] / counts[present]).sum())
    W = float((counts * np.square(c64).sum(axis=1)).sum())
    loss = 0.5 / B * (S1 - 0.5 * A - 0.75 * X + 0.25 * W)
    return np.float32(loss), res


def _kernel_fallback(feats, labels_i, centers, trace=False):
    nc = _get_program("fallback")
    _CACHE["nc"] = nc
    in_maps = []
    for c in range(N_CORES):
        fshard = np.ascontiguousarray(feats[c * BS : (c + 1) * BS])
        lshard = labels_i[c * BS : (c + 1) * BS]
        ltile = np.ascontiguousarray(lshard.reshape(TILES, P).T.astype(np.float16))
        in_maps.append({"feats": fshard, "labels": ltile})
    res = _run_spmd(nc, in_maps, trace=trace)

    sums_all = np.zeros((CPAD, NFREE), dtype=np.float64)
    S1 = 0.0
    for c in range(N_CORES):
        raw = res.results[c]["out_sums"]
        part = (
            raw[:, : CCHUNKS * NFREE]
            .reshape(P, CCHUNKS, NFREE)
            .transpose(1, 0, 2)
            .reshape(CPAD, NFREE)
        )
        sums_all += part.astype(np.float64)
        S1 += float(raw[:, CCHUNKS * NFREE].sum())
    sums = sums_all[:C, :D]
    counts = sums_all[:C, D]

    c64 = centers.astype(np.float64)
    A = float((sums * c64).sum())
    present = counts > 0
    X = float((np.square(sums).sum(axis=1)[present] / counts[present]).sum())
    W = float((counts * np.square(c64).sum(axis=1)).sum())
    loss = 0.5 / B * (S1 - 0.5 * A - 0.75 * X + 0.25 * W)
    return np.float32(loss), res


def kernel(feats, centers, labels, _trace: bool = False, _return_res: bool = False):
    feats = np.asarray(feats, dtype=np.float32)
    centers = np.asarray(centers, dtype=np.float32)
    labels_i = np.asarray(labels).astype(np.int64)

    split = _class_split(labels_i)
    if split is not None:
        order, windows = split
        out, res = _kernel_fast(feats, centers, labels_i, order, windows,
                                trace=_trace)
    else:
        out, res = _kernel_fallback(feats, labels_i, centers, trace=_trace)
    if _return_res:
        return out, res
    return out
